# revision 1
# baseline (speedup 1.0000x reference)
# Trainium2 Bass kernel for nn_CrossFrequencyInteraction.
#
# Reference computation (per batch item, two symmetric branches):
#   q = Wq @ x_q;  k = Wk @ x_kv;  v = Wv @ x_kv          (1x1 convs, C=256)
#   out = softmax_n(q) used against ctx = softmax_n(k) @ v^T   (linear attention)
#   inter = Wp @ out;  x_q += inter
#   then training-mode BatchNorm over (B,H,W) on both updated rgb tensors.
#
# Sharding: data-parallel over batch (B=8 -> 1 item per core, 8 cores).
# BN statistics (per-channel sum/sumsq) are AllReduced across cores (2KB).
#
# Key algebraic restructurings (all exact):
#   - b_q, b_k shift softmax inputs by a per-row constant along the softmax
#     axis -> they cancel exactly; skipped.
#   - b_proj is a per-channel constant shift -> absorbed exactly by BN; skipped.
#   - b_v adds b_v[e] to ctx[d,e] (softmax_k sums to 1) -> folded into ctx.
#   - softmax normalizers (1/sum exp) for q and k are per-channel scales that
#     commute through the attention contraction -> folded into the tiny
#     M = Wp . blockdiag(ctx^T) matrix, so attention-out + proj become a
#     single [256,256] @ [256,4096] matmul per branch.
#   - kT/vT are produced directly in transposed layout by using x as the
#     stationary matmul operand (no explicit transposes anywhere); the
#     softmax-k denominators come for free from a ones-column in vT.

import os
import numpy as np

C = 256
N = 4096
NBLK = 2          # channel blocks of 128
NT = 32           # n-tiles of 128 (for transposed convs)
NCH = 8           # n-chunks of 512 (for natural convs)
NH = 4            # heads
HD = 64           # head dim
NCORES = 8
BHW = 8 * 64 * 64  # BN reduction count
EPS = 1e-5

_CACHE = {}


def _build():
    import concourse.bass as bass
    import concourse.bacc as bacc
    import concourse.tile as tile
    from concourse import mybir
    from contextlib import ExitStack

    F32 = mybir.dt.float32
    F32R = mybir.dt.float32r
    BF16 = mybir.dt.bfloat16
    OP = mybir.AluOpType
    AF = mybir.ActivationFunctionType
    AX = mybir.AxisListType

    nc = bacc.Bacc("TRN2", num_devices=NCORES)

    xq_d = [nc.dram_tensor(n_, [C, N], F32, kind="ExternalInput")
            for n_ in ("xq1", "xq2")]
    xkv_d = [nc.dram_tensor(n_, [C, N], F32, kind="ExternalInput")
             for n_ in ("xkv1", "xkv2")]
    # wt: [256, 8*256] = [Wq1^T|Wk1^T|Wv1^T|Wp1^T|Wq2^T|Wk2^T|Wv2^T|Wp2^T]
    wt_d = nc.dram_tensor("wt", [C, 8 * 256], BF16, kind="ExternalInput")
    # wp: [64, 8*256]; block (b*4+h) = Wp_b^T[h*64:(h+1)*64, :]
    wp_d = nc.dram_tensor("wp", [64, 8 * 256], BF16, kind="ExternalInput")
    # bp: [128, 8] cols = (bv1_b0, bv1_b1, bv2_b0, bv2_b1, g_b0, g_b1, be_b0, be_b1)
    bp_d = nc.dram_tensor("bp", [128, 8], F32, kind="ExternalInput")
    # bh: [64, 8]; col (b*4+h) = b_v[h*64:(h+1)*64] for branch b
    bh_d = nc.dram_tensor("bh", [64, 8], F32, kind="ExternalInput")
    out_d = [nc.dram_tensor(n_, [C, N], F32, kind="ExternalOutput")
             for n_ in ("out1", "out2")]

    with ExitStack() as ctx:
        tc = ctx.enter_context(tile.TileContext(nc))
        const = ctx.enter_context(tc.tile_pool(name="const", bufs=1))
        xqp = ctx.enter_context(tc.tile_pool(name="xqp", bufs=1))
        xkvp = ctx.enter_context(tc.tile_pool(name="xkvp", bufs=1))
        eqp = ctx.enter_context(tc.tile_pool(name="eqp", bufs=1))
        ekp = ctx.enter_context(tc.tile_pool(name="ekp", bufs=6))
        vtp = ctx.enter_context(tc.tile_pool(name="vtp", bufs=6))
        misc = ctx.enter_context(tc.tile_pool(name="misc", bufs=1))
        scr = ctx.enter_context(tc.tile_pool(name="scr", bufs=2))
        bigp = ctx.enter_context(tc.tile_pool(name="bigp", bufs=4, space="PSUM"))
        smallp = ctx.enter_context(tc.tile_pool(name="smallp", bufs=4, space="PSUM"))
        dramp = ctx.enter_context(tc.tile_pool(name="dramp", bufs=1, space="DRAM"))

        # ---- constants ----
        wt_sb = []
        for k in range(NBLK):
            w = const.tile([128, 8 * 256], BF16, name=f"wt{k}", tag=f"wt{k}")
            nc.sync.dma_start(out=w, in_=wt_d[k * 128:(k + 1) * 128, :])
            wt_sb.append(w)
        wp_sb = const.tile([64, 8 * 256], BF16, name="wp", tag="wp")
        nc.sync.dma_start(out=wp_sb, in_=wp_d[:, :])
        bp_sb = const.tile([128, 8], F32, name="bp", tag="bp")
        nc.sync.dma_start(out=bp_sb, in_=bp_d[:, :])
        bh_sb = const.tile([64, 8], F32, name="bh", tag="bh")
        nc.sync.dma_start(out=bh_sb, in_=bh_d[:, :])
        ones_col = const.tile([128, 1], BF16, name="ones_col", tag="ones_col")
        nc.vector.memset(ones_col, 1.0)

        expq = [eqp.tile([128, N], BF16, name=f"expq{k}", tag=f"expq{k}")
                for k in range(NBLK)]

        rg = [list(range(NCORES))]

        ablate = os.environ.get("KERNEL_ABLATE", "")

        def branch(b):
            wofs = 4 * b
            # ---- load inputs ----
            xkv = []
            for k in range(NBLK):
                t = xkvp.tile([128, N], BF16, name=f"xkv{k}_b{b}", tag=f"xkv{k}")
                nc.gpsimd.dma_start(out=t, in_=xkv_d[b][k * 128:(k + 1) * 128, :])
                xkv.append(t)
            xq = []
            xqb = []
            for k in range(NBLK):
                t = xqp.tile([128, N], F32, name=f"xq{k}_b{b}", tag=f"xq{k}_b{b}")
                nc.sync.dma_start(out=t, in_=xq_d[b][k * 128:(k + 1) * 128, :])
                xq.append(t)
                tb = xqp.tile([128, N], BF16, name=f"xqb{k}_b{b}", tag=f"xqb{k}")
                nc.vector.tensor_copy(tb, t)
                xqb.append(tb)

            # ---- KV phase: kT|vT transposed convs + exp(k) + ctx accumulation
            pctx = smallp.tile([128, 256], F32, name=f"pctx_b{b}", tag="small")
            pden = smallp.tile([1, 256], F32, name=f"pden_b{b}", tag="small")
            wkv0 = (wofs + 1) * 256
            for t in range(NT):
                pkv = bigp.tile([128, 512], F32, name=f"pkv_b{b}_{t}", tag="big")
                for k in range(NBLK):
                    nc.tensor.matmul(
                        pkv,
                        lhsT=xkv[k][:, t * 128:(t + 1) * 128],
                        rhs=wt_sb[k][:, wkv0:wkv0 + 512],
                        start=(k == 0), stop=(k == NBLK - 1),
                    )
                ek = ekp.tile([128, 256], BF16, name=f"ek_b{b}_{t}", tag="ek")
                nc.scalar.activation(ek, pkv[:, 0:256], AF.Exp)
                vt = vtp.tile([128, 256], BF16, name=f"vt_b{b}_{t}", tag="vt")
                nc.vector.tensor_copy(vt, pkv[:, 256:512])
                for h in range(NH):
                    nc.tensor.matmul(
                        pctx[0:HD, h * HD:(h + 1) * HD],
                        lhsT=vt[:, h * HD:(h + 1) * HD],
                        rhs=ek[:, h * HD:(h + 1) * HD],
                        start=(t == 0), stop=(t == NT - 1),
                        skip_group_check=True,
                    )
                nc.tensor.matmul(
                    pden, lhsT=ones_col, rhs=ek,
                    start=(t == 0), stop=(t == NT - 1),
                    skip_group_check=True,
                )

            # ---- ctx eviction (+ b_v fold) and softmax-k denominators ----
            ctxT = misc.tile([64, 256], BF16, name=f"ctxT_b{b}", tag="ctxT")
            for h in range(NH):
                nc.vector.tensor_scalar(
                    ctxT[:, h * HD:(h + 1) * HD],
                    pctx[0:HD, h * HD:(h + 1) * HD],
                    bh_sb[:, b * 4 + h:b * 4 + h + 1],
                    None, OP.add)
            drow = misc.tile([1, 256], F32, name=f"drow_b{b}", tag="drow")
            nc.vector.tensor_copy(drow, pden)
            denT = misc.tile([128, 2], F32, name=f"denT_b{b}", tag="denT")
            for k in range(NBLK):
                nc.sync.dma_start(out=denT[:, k:k + 1],
                                  in_=drow[0:1, k * 128:(k + 1) * 128])
            if ablate.endswith("kv"):
                ctf = misc.tile([64, 256], F32, name=f"ctf_b{b}", tag="ctf")
                nc.vector.tensor_copy(ctf, ctxT)
                nc.sync.dma_start(out=out_d[b][0:64, 0:256], in_=ctf)
                nc.sync.dma_start(out=out_d[b][64:128, 0:2], in_=denT[0:64, :])
                return

            # ---- Q phase: natural conv + exp with accumulated row sums ----
            sqp = misc.tile([128, NBLK, NCH], F32, name=f"sqp_b{b}", tag="sqp")
            wq0 = wofs * 256
            for k in range(NBLK):
                for j in range(NCH):
                    pq = bigp.tile([128, 512], F32, name=f"pq_b{b}_{k}_{j}", tag="big")
                    for kk in range(NBLK):
                        nc.tensor.matmul(
                            pq,
                            lhsT=wt_sb[kk][:, wq0 + k * 128:wq0 + k * 128 + 128],
                            rhs=xqb[kk][:, j * 512:(j + 1) * 512],
                            start=(kk == 0), stop=(kk == NBLK - 1),
                        )
                    nc.scalar.activation(
                        expq[k][:, j * 512:(j + 1) * 512], pq, AF.Exp,
                        accum_out=sqp[:, k, j:j + 1])

            # ---- normalization factor: 1 / (denom_k * sum_q) ----
            sq2 = misc.tile([128, 2], F32, name=f"sq2_b{b}", tag="sq2")
            for k in range(NBLK):
                nc.vector.reduce_sum(sq2[:, k:k + 1], sqp[:, k, :], axis=AX.X)
            fde = misc.tile([128, 2], F32, name=f"fde_b{b}", tag="fde")
            nc.vector.tensor_mul(fde, denT, sq2)
            fac = misc.tile([128, 2], F32, name=f"fac_b{b}", tag="fac")
            nc.vector.reciprocal(fac, fde)

            # ---- M^T: per-head Wp . ctx^T with folded normalization ----
            mt_sb = []
            for p in range(2):
                pmt = smallp.tile([128, 256], F32, name=f"pmt_b{b}_{p}", tag="small")
                for hh in range(2):
                    h = p * 2 + hh
                    nc.tensor.matmul(
                        pmt[hh * 64:(hh + 1) * 64, :],
                        lhsT=ctxT[:, h * HD:(h + 1) * HD],
                        rhs=wp_sb[:, (b * 4 + h) * 256:(b * 4 + h + 1) * 256],
                        start=True, stop=True,
                        tile_position=(0, hh * 64),
                    )
                mt = misc.tile([128, 256], BF16, name=f"mt_b{b}_{p}", tag=f"mt{p}")
                nc.vector.tensor_scalar(mt, pmt, fac[:, p:p + 1], None, OP.mult)
                mt_sb.append(mt)
            if ablate.endswith("q"):
                for p in range(2):
                    mtf = misc.tile([128, 256], F32, name=f"mtf_b{b}_{p}", tag="mtf")
                    nc.vector.tensor_copy(mtf, mt_sb[p])
                    nc.sync.dma_start(out=out_d[b][p * 128:(p + 1) * 128, 0:256], in_=mtf)
                return

            # ---- inter = M^T.T @ expq, fused residual + BN stats ----
            bst = misc.tile([128, NBLK, NCH, 6], F32, name=f"bst_b{b}", tag="bst")
            for k in range(NBLK):
                for j in range(NCH):
                    pi = bigp.tile([128, 512], F32, name=f"pi_b{b}_{k}_{j}", tag="big")
                    for kk in range(NBLK):
                        nc.tensor.matmul(
                            pi,
                            lhsT=mt_sb[kk][:, k * 128:(k + 1) * 128],
                            rhs=expq[kk][:, j * 512:(j + 1) * 512],
                            start=(kk == 0), stop=(kk == NBLK - 1),
                        )
                    xs = xq[k][:, j * 512:(j + 1) * 512]
                    nc.vector.scalar_tensor_tensor(
                        xs, pi, 1.0, xs, OP.mult, OP.add)
                    nc.vector.bn_stats(bst[:, k, j, :], xs)

            if ablate.endswith("i"):
                for k in range(NBLK):
                    nc.sync.dma_start(out=out_d[b][k * 128:(k + 1) * 128, :],
                                      in_=xq[k])
                return

            # ---- BN stats allreduce ----
            cc_sb = misc.tile([128, 4], F32, name=f"cc_b{b}", tag=f"cc{b}")
            mv = misc.tile([128, NBLK, 2], F32, name=f"mv_b{b}", tag="mv")
            for k in range(NBLK):
                nc.vector.bn_aggr(mv[:, k, :], bst[:, k, :, :])
                # sum = mean * N ; sumsq = (var + mean^2) * N   (per-core N)
                nc.vector.tensor_scalar(
                    cc_sb[:, 2 * k:2 * k + 1], mv[:, k, 0:1], float(N), None, OP.mult)
                nc.vector.scalar_tensor_tensor(
                    cc_sb[:, 2 * k + 1:2 * k + 2], mv[:, k, 0:1],
                    mv[:, k, 0:1], mv[:, k, 1:2], OP.mult, OP.add)
                nc.vector.tensor_scalar(
                    cc_sb[:, 2 * k + 1:2 * k + 2], cc_sb[:, 2 * k + 1:2 * k + 2],
                    float(N), None, OP.mult)
            ccr = misc.tile([128, 4], F32, name=f"ccr_b{b}", tag=f"ccr{b}")
            if os.environ.get("KERNEL_ABLATE") == "nocc":
                nc.vector.tensor_scalar(ccr, cc_sb, float(NCORES), None, OP.mult)
            else:
                cc_in = dramp.tile([128, 4], F32, name=f"ccin_b{b}", tag=f"ccin{b}")
                cc_out = dramp.tile([128, 4], F32, name=f"ccout_b{b}", tag=f"ccout{b}",
                                    addr_space="Shared")
                nc.sync.dma_start(out=cc_in, in_=cc_sb)
                nc.gpsimd.collective_compute(
                    "AllReduce", OP.add, replica_groups=rg,
                    ins=[cc_in[:, :]], outs=[cc_out[:, :]])
                nc.sync.dma_start(out=ccr, in_=cc_out)

            # ---- BN affine coefficients ----
            ccr3 = ccr.rearrange("p (k s) -> p k s", k=2)
            mean = misc.tile([128, 2], F32, name=f"mean_b{b}", tag="mean")
            nc.vector.tensor_scalar(mean, ccr3[:, :, 0], 1.0 / BHW, None, OP.mult)
            var = misc.tile([128, 2], F32, name=f"var_b{b}", tag="var")
            # var = sumsq/BHW - mean^2
            m2 = misc.tile([128, 2], F32, name=f"m2_b{b}", tag="m2")
            nc.vector.tensor_mul(m2, mean, mean)
            nc.vector.scalar_tensor_tensor(
                var, ccr3[:, :, 1], 1.0 / BHW, m2, OP.mult, OP.subtract)
            sd = misc.tile([128, 2], F32, name=f"sd_b{b}", tag="sd")
            epst = misc.tile([128, 1], F32, name=f"eps_b{b}", tag="eps")
            nc.vector.memset(epst, EPS)
            nc.scalar.activation(sd, var, AF.Sqrt, bias=epst)
            rs = misc.tile([128, 2], F32, name=f"rs_b{b}", tag="rs")
            nc.vector.reciprocal(rs, sd)
            s2 = misc.tile([128, 2], F32, name=f"s2_b{b}", tag="s2")
            nc.vector.tensor_mul(s2, rs, bp_sb[:, 4:6])
            ms = misc.tile([128, 2], F32, name=f"ms_b{b}", tag="ms")
            nc.vector.tensor_mul(ms, mean, s2)
            t2 = misc.tile([128, 2], F32, name=f"t2_b{b}", tag="t2")
            nc.vector.tensor_sub(t2, bp_sb[:, 6:8], ms)

            # ---- normalize in place and store, chunked for overlap ----
            for k in range(NBLK):
                for jc in range(4):
                    sl = slice(jc * 1024, (jc + 1) * 1024)
                    nc.vector.tensor_scalar(
                        xq[k][:, sl], xq[k][:, sl],
                        s2[:, k:k + 1], t2[:, k:k + 1], OP.mult, OP.add)
                    nc.sync.dma_start(
                        out=out_d[b][k * 128:(k + 1) * 128, sl],
                        in_=xq[k][:, sl])

        branch(0)
        if not ablate.startswith("b0"):
            branch(1)

    nc.finalize()
    return nc


def _get_nc():
    if "nc" not in _CACHE:
        _CACHE["nc"] = _build()
    return _CACHE["nc"]


def _pack_host(inputs):
    import ml_dtypes
    bf16 = ml_dtypes.bfloat16
    f32 = np.float32
    ws = []
    for b in ("1", "2"):
        for w in ("q", "k", "v", "proj"):
            ws.append(np.ascontiguousarray(
                np.asarray(inputs[f"w_{w}{b}"], dtype=f32).T))
    wt = np.concatenate(ws, axis=1).astype(bf16)  # [256, 2048]

    wps = []
    for b in ("1", "2"):
        wpT = np.ascontiguousarray(np.asarray(inputs[f"w_proj{b}"], dtype=f32).T)
        for h in range(NH):
            wps.append(wpT[h * HD:(h + 1) * HD, :])
    wp = np.concatenate(wps, axis=1).astype(bf16)  # [64, 2048]

    bv1 = np.asarray(inputs["b_v1"], dtype=f32)
    bv2 = np.asarray(inputs["b_v2"], dtype=f32)
    g = np.asarray(inputs["gamma"], dtype=f32)
    be = np.asarray(inputs["beta"], dtype=f32)
    bp = np.stack([bv1[:128], bv1[128:], bv2[:128], bv2[128:],
                   g[:128], g[128:], be[:128], be[128:]], axis=1)  # [128, 8]
    bh = np.stack([bv1[h * HD:(h + 1) * HD] for h in range(NH)]
                  + [bv2[h * HD:(h + 1) * HD] for h in range(NH)], axis=1)  # [64, 8]
    return (np.ascontiguousarray(wt), np.ascontiguousarray(wp),
            np.ascontiguousarray(bp), np.ascontiguousarray(bh))


def kernel(rgb_low, rgb_high, dsm_low, dsm_high,
           w_q1, b_q1, w_k1, b_k1, w_v1, b_v1,
           w_q2, b_q2, w_k2, b_k2, w_v2, b_v2,
           w_proj1, b_proj1, w_proj2, b_proj2, gamma, beta,
           _trace=False):
    from concourse.bass_utils import run_bass_kernel_spmd

    inputs = dict(rgb_low=rgb_low, rgb_high=rgb_high, dsm_low=dsm_low,
                  dsm_high=dsm_high, w_q1=w_q1, w_k1=w_k1, w_v1=w_v1,
                  w_proj1=w_proj1, w_q2=w_q2, w_k2=w_k2, w_v2=w_v2,
                  w_proj2=w_proj2, b_v1=b_v1, b_v2=b_v2, gamma=gamma, beta=beta)
    f32 = np.float32
    rl = np.asarray(rgb_low, dtype=f32)
    rh = np.asarray(rgb_high, dtype=f32)
    dl = np.asarray(dsm_low, dtype=f32)
    dh = np.asarray(dsm_high, dtype=f32)
    B = rl.shape[0]
    assert B == NCORES, f"expected batch {NCORES}, got {B}"

    wt, wp, bp, bh = _pack_host(inputs)
    nc = _get_nc()

    in_maps = []
    for i in range(NCORES):
        in_maps.append({
            "xq1": np.ascontiguousarray(rl[i].reshape(C, N)),
            "xkv1": np.ascontiguousarray(dh[i].reshape(C, N)),
            "xq2": np.ascontiguousarray(rh[i].reshape(C, N)),
            "xkv2": np.ascontiguousarray(dl[i].reshape(C, N)),
            "wt": wt, "wp": wp, "bp": bp, "bh": bh,
        })

    res = run_bass_kernel_spmd(nc, in_maps, core_ids=list(range(NCORES)),
                               trace=_trace)
    out_low = np.stack([res.results[i]["out1"].reshape(C, 64, 64)
                        for i in range(NCORES)])
    out_high = np.stack([res.results[i]["out2"].reshape(C, 64, 64)
                         for i in range(NCORES)])
    if _trace:
        _CACHE["last_results"] = res
    return (out_low, out_high, np.asarray(dsm_low), np.asarray(dsm_high))



# revision 5
# speedup vs baseline: 1.1000x; 1.1000x over previous
# Trainium2 Bass kernel for nn_CrossFrequencyInteraction.
#
# Reference computation (per batch item, two symmetric branches):
#   q = Wq @ x_q;  k = Wk @ x_kv;  v = Wv @ x_kv          (1x1 convs, C=256)
#   out = softmax_n(q) used against ctx = softmax_n(k) @ v^T   (linear attention)
#   inter = Wp @ out;  x_q += inter
#   then training-mode BatchNorm over (B,H,W) on both updated rgb tensors.
#
# Sharding: data-parallel over batch (B=8 -> 1 item per core, 8 cores).
# BN statistics (per-channel sum/sumsq) are AllReduced across cores (2KB).
#
# Key algebraic restructurings (all exact):
#   - b_q, b_k shift softmax inputs by a per-row constant along the softmax
#     axis -> they cancel exactly; skipped.
#   - b_proj is a per-channel constant shift -> absorbed exactly by BN; skipped.
#   - b_v adds b_v[e] to ctx[d,e] (softmax_k sums to 1) -> folded into ctx.
#   - softmax normalizers (1/sum exp) for q and k are per-channel scales that
#     commute through the attention contraction -> folded into the tiny
#     M = Wp . blockdiag(ctx^T) matrix, so attention-out + proj become a
#     single [256,256] @ [256,4096] matmul per branch.
#   - kT/vT are produced directly in transposed layout by using x as the
#     stationary matmul operand (no explicit transposes anywhere); the
#     softmax-k denominators come from batched ones-row matmuls over paired
#     exp(k) tiles.
#
# Performance structure:
#   - ctx accumulation uses head-PAIR matmuls ([128,128] with garbage in the
#     cross-head blocks) into a [128,256] PSUM tile; evictions take only the
#     diagonal blocks into pre-zeroed block-diagonal SBUF tiles, so M^T is
#     2 matmuls per branch instead of 8 and ctx is 2 matmuls/tile not 4.
#   - x_q is loaded once, as bf16, via casting DMA (no Vector casts); the
#     residual x+inter is held in bf16 (error ~4e-4 << 2e-2 gate).
#   - PSUM->SBUF v^T evictions and the residual add run on GpSimd, BN stats
#     on Vector, exps on Scalar: no engine is oversubscribed.
#   - Branch-1's normalize+store is fenced to start only after branch-2's BN
#     stats are ready, so it fills the ~26us dead window of branch-2's
#     AllReduce instead of stealing Vector time from branch-2's compute.
#   - Input DMAs are chunked so the first KV matmul starts ~2MB into the
#     load, and the KV loop is software-pipelined (pkv(t+1) before ctx(t)).

import os
import numpy as np

C = 256
N = 4096
NBLK = 2          # channel blocks of 128
NT = 32           # n-tiles of 128 (for transposed convs)
NPAIR = NT // 2   # exp(k) tile pairs
NCH = 8           # n-chunks of 512 (for natural convs)
NH = 4            # heads
HD = 64           # head dim
NCORES = 8
BHW = 8 * 64 * 64  # BN reduction count
EPS = 1e-5
NLC = 4           # DMA load chunks per [128, N] input tile
NSC = 4           # store chunks of 1024 per channel block

_CACHE = {}


def _build():
    import concourse.bass as bass
    import concourse.bacc as bacc
    import concourse.tile as tile
    from concourse import mybir
    from contextlib import ExitStack

    F32 = mybir.dt.float32
    BF16 = mybir.dt.bfloat16
    OP = mybir.AluOpType
    AF = mybir.ActivationFunctionType
    AX = mybir.AxisListType

    nc = bacc.Bacc("TRN2", num_devices=NCORES)

    xq_d = [nc.dram_tensor(n_, [C, N], F32, kind="ExternalInput")
            for n_ in ("xq1", "xq2")]
    xkv_d = [nc.dram_tensor(n_, [C, N], F32, kind="ExternalInput")
             for n_ in ("xkv1", "xkv2")]
    # wt: [256, 6*256] = [Wq1^T|Wk1^T|Wv1^T|Wq2^T|Wk2^T|Wv2^T]
    wt_d = nc.dram_tensor("wt", [C, 6 * 256], BF16, kind="ExternalInput")
    # wp: [128, 4*256]; block (2b+p) = Wp_b^T[p*128:(p+1)*128, :]
    wp_d = nc.dram_tensor("wp", [128, 4 * 256], BF16, kind="ExternalInput")
    # bp: [128, 4] cols = (g_b0, g_b1, be_b0, be_b1)
    bp_d = nc.dram_tensor("bp", [128, 4], F32, kind="ExternalInput")
    # bh: [128, 4]; col (2b+p) = b_v[branch b][p*128:(p+1)*128]
    bh_d = nc.dram_tensor("bh", [128, 4], F32, kind="ExternalInput")
    out_d = [nc.dram_tensor(n_, [C, N], F32, kind="ExternalOutput")
             for n_ in ("out1", "out2")]

    with ExitStack() as ctx:
        tc = ctx.enter_context(tile.TileContext(nc))
        const = ctx.enter_context(tc.tile_pool(name="const", bufs=1))
        xqp = ctx.enter_context(tc.tile_pool(name="xqp", bufs=1))
        xkvp = ctx.enter_context(tc.tile_pool(name="xkvp", bufs=1))
        xsp = ctx.enter_context(tc.tile_pool(name="xsp", bufs=1))
        eqp = ctx.enter_context(tc.tile_pool(name="eqp", bufs=1))
        ekp = ctx.enter_context(tc.tile_pool(name="ekp", bufs=4))
        vtp = ctx.enter_context(tc.tile_pool(name="vtp", bufs=4))
        misc = ctx.enter_context(tc.tile_pool(name="misc", bufs=1))
        stg = ctx.enter_context(tc.tile_pool(name="stg", bufs=4))
        bigp = ctx.enter_context(tc.tile_pool(name="bigp", bufs=4, space="PSUM"))
        smallp = ctx.enter_context(tc.tile_pool(name="smallp", bufs=4, space="PSUM"))
        dramp = ctx.enter_context(tc.tile_pool(name="dramp", bufs=1, space="DRAM"))

        # ---- constants ----
        wt_sb = []
        for k in range(NBLK):
            w = const.tile([128, 6 * 256], BF16, name=f"wt{k}", tag=f"wt{k}")
            nc.sync.dma_start(out=w, in_=wt_d[k * 128:(k + 1) * 128, :])
            wt_sb.append(w)
        wp_sb = const.tile([128, 4 * 256], BF16, name="wp", tag="wp")
        nc.sync.dma_start(out=wp_sb, in_=wp_d[:, :])
        bp_sb = const.tile([128, 4], F32, name="bp", tag="bp")
        nc.sync.dma_start(out=bp_sb, in_=bp_d[:, :])
        bh_sb = const.tile([128, 4], F32, name="bh", tag="bh")
        nc.sync.dma_start(out=bh_sb, in_=bh_d[:, :])
        ones_col = const.tile([128, 1], BF16, name="ones_col", tag="ones_col")
        nc.vector.memset(ones_col, 1.0)
        epst = const.tile([128, 1], F32, name="epst", tag="epst")
        nc.vector.memset(epst, EPS)
        # pre-zeroed block-diagonal ctx^T tiles (only diag blocks ever written)
        ctxq = []
        for i in range(4):
            t = const.tile([128, 128], BF16, name=f"ctxq{i}", tag=f"ctxq{i}")
            nc.vector.memset(t, 0.0)
            ctxq.append(t)

        expq = [eqp.tile([128, N], BF16, name=f"expq{k}", tag=f"expq{k}")
                for k in range(NBLK)]

        rg = [list(range(NCORES))]

        def load_inputs(b):
            xkv, xqb = [], []
            for k in range(NBLK):
                t = xkvp.tile([128, N], BF16, name=f"xkv{k}_b{b}", tag=f"xkv{k}_b{b}")
                for jc in range(NLC):
                    sl = slice(jc * (N // NLC), (jc + 1) * (N // NLC))
                    nc.gpsimd.dma_start(out=t[:, sl],
                                        in_=xkv_d[b][k * 128:(k + 1) * 128, sl])
                xkv.append(t)
            for k in range(NBLK):
                t = xqp.tile([128, N], BF16, name=f"xqb{k}_b{b}", tag=f"xqb{k}_b{b}")
                for jc in range(NLC):
                    sl = slice(jc * (N // NLC), (jc + 1) * (N // NLC))
                    nc.gpsimd.dma_start(out=t[:, sl],
                                        in_=xq_d[b][k * 128:(k + 1) * 128, sl])
                xqb.append(t)
            return xkv, xqb

        def branch_compute(b, xkv, xqb):
            wq0 = b * 768
            wkv0 = b * 768 + 256

            # ---- KV phase: kT|vT transposed convs + exp(k) + ctx/den accum
            pctx = smallp.tile([128, 256], F32, name=f"pctx_b{b}", tag="small")
            pden = smallp.tile([1, 512], F32, name=f"pden_b{b}", tag="small")
            ekw = [None] * NPAIR
            vts = [None] * NT

            def kv_front(t):
                pair, half = t // 2, t % 2
                if half == 0:
                    ekw[pair] = ekp.tile([128, 512], BF16,
                                         name=f"ekw_b{b}_{pair}", tag="ekw")
                pkv = bigp.tile([128, 512], F32, name=f"pkv_b{b}_{t}", tag="big")
                for k in range(NBLK):
                    nc.tensor.matmul(
                        pkv,
                        lhsT=xkv[k][:, t * 128:(t + 1) * 128],
                        rhs=wt_sb[k][:, wkv0:wkv0 + 512],
                        start=(k == 0), stop=(k == NBLK - 1))
                nc.scalar.activation(ekw[pair][:, half * 256:(half + 1) * 256],
                                     pkv[:, 0:256], AF.Exp)
                vt = vtp.tile([128, 256], BF16, name=f"vt_b{b}_{t}", tag="vt")
                nc.vector.tensor_copy(vt, pkv[:, 256:512])
                vts[t] = vt

            def kv_ctx(t):
                pair, half = t // 2, t % 2
                base = half * 256
                for p in range(2):
                    nc.tensor.matmul(
                        pctx[:, p * 128:(p + 1) * 128],
                        lhsT=vts[t][:, p * 128:(p + 1) * 128],
                        rhs=ekw[pair][:, base + p * 128:base + (p + 1) * 128],
                        start=(t == 0), stop=(t == NT - 1),
                        skip_group_check=True)
                if half == 1:
                    nc.tensor.matmul(
                        pden, lhsT=ones_col, rhs=ekw[pair][:, :],
                        start=(pair == 0), stop=(pair == NPAIR - 1),
                        skip_group_check=True)

            kv_front(0)
            for t in range(1, NT):
                kv_front(t)
                kv_ctx(t - 1)
            kv_ctx(NT - 1)

            # ---- softmax-k denominators: even+odd halves, transpose to cols
            drow_sb = misc.tile([1, 512], F32, name=f"drow_sb_b{b}", tag="drow_sb")
            nc.vector.tensor_copy(drow_sb, pden)
            drow = misc.tile([1, 256], F32, name=f"drow_b{b}", tag="drow")
            nc.vector.tensor_add(drow, drow_sb[0:1, 0:256], drow_sb[0:1, 256:512])
            denT = misc.tile([128, 2], F32, name=f"denT_b{b}", tag="denT")
            for k in range(NBLK):
                nc.scalar.dma_start(out=denT[:, k:k + 1],
                                    in_=drow[0:1, k * 128:(k + 1) * 128])

            # ---- ctx eviction (diag blocks only) with b_v fold ----
            for p in range(2):
                cq = ctxq[2 * b + p]
                col = 2 * b + p
                nc.vector.tensor_scalar(
                    cq[0:64, 0:64], pctx[0:64, p * 128:p * 128 + 64],
                    bh_sb[0:64, col:col + 1], None, OP.add)
                nc.vector.tensor_scalar(
                    cq[64:128, 64:128], pctx[64:128, p * 128 + 64:(p + 1) * 128],
                    bh_sb[64:128, col:col + 1], None, OP.add)

            # ---- M^T per head pair (tiny; PE runs these during Q phase) ----
            pmt = [smallp.tile([128, 256], F32, name=f"pmt_b{b}_{p}", tag="small")
                   for p in range(2)]
            for p in range(2):
                nc.tensor.matmul(
                    pmt[p], lhsT=ctxq[2 * b + p][:, :],
                    rhs=wp_sb[:, (2 * b + p) * 256:(2 * b + p + 1) * 256],
                    start=True, stop=True)

            # ---- Q phase: natural conv + exp with accumulated row sums ----
            sqp = misc.tile([128, NBLK, NCH], F32, name=f"sqp_b{b}", tag="sqp")
            sq2 = misc.tile([128, 2], F32, name=f"sq2_b{b}", tag="sq2")
            fde = misc.tile([128, 2], F32, name=f"fde_b{b}", tag="fde")
            fac = misc.tile([128, 2], F32, name=f"fac_b{b}", tag="fac")
            mt_sb = []
            for k in range(NBLK):
                for j in range(NCH):
                    pq = bigp.tile([128, 512], F32, name=f"pq_b{b}_{k}_{j}", tag="big")
                    for kk in range(NBLK):
                        nc.tensor.matmul(
                            pq,
                            lhsT=wt_sb[kk][:, wq0 + k * 128:wq0 + k * 128 + 128],
                            rhs=xqb[kk][:, j * 512:(j + 1) * 512],
                            start=(kk == 0), stop=(kk == NBLK - 1))
                    nc.scalar.activation(
                        expq[k][:, j * 512:(j + 1) * 512], pq, AF.Exp,
                        accum_out=sqp[:, k, j:j + 1])
                # normalization factor for this block: 1 / (denom_k * sum_q)
                nc.vector.reduce_sum(sq2[:, k:k + 1], sqp[:, k, :], axis=AX.X)
                nc.vector.tensor_mul(fde[:, k:k + 1], denT[:, k:k + 1],
                                     sq2[:, k:k + 1])
                nc.vector.reciprocal(fac[:, k:k + 1], fde[:, k:k + 1])
                mt = misc.tile([128, 256], BF16, name=f"mt_b{b}_{k}", tag=f"mt{k}")
                nc.vector.tensor_scalar(mt, pmt[k], fac[:, k:k + 1], None, OP.mult)
                mt_sb.append(mt)

            # ---- inter = M^T.T @ expq, residual into bf16 xs, BN stats ----
            xs = [xsp.tile([128, N], BF16, name=f"xs{k}_b{b}", tag=f"xs{k}_b{b}")
                  for k in range(NBLK)]
            bst = misc.tile([128, NBLK, NCH, 6], F32, name=f"bst_b{b}", tag="bst")
            for k in range(NBLK):
                for j in range(NCH):
                    pi = bigp.tile([128, 512], F32, name=f"pi_b{b}_{k}_{j}", tag="big")
                    for kk in range(NBLK):
                        nc.tensor.matmul(
                            pi,
                            lhsT=mt_sb[kk][:, k * 128:(k + 1) * 128],
                            rhs=expq[kk][:, j * 512:(j + 1) * 512],
                            start=(kk == 0), stop=(kk == NBLK - 1))
                    sl = slice(j * 512, (j + 1) * 512)
                    nc.vector.scalar_tensor_tensor(
                        xs[k][:, sl], pi, 1.0, xqb[k][:, sl], OP.mult, OP.add)
                    nc.vector.bn_stats(bst[:, k, j, :], xs[k][:, sl])

            # ---- local stats -> (sum, sumsq), launch allreduce ----
            cc_sb = misc.tile([128, 4], F32, name=f"cc_b{b}", tag=f"cc{b}")
            mv = misc.tile([128, NBLK, 2], F32, name=f"mv_b{b}", tag=f"mv{b}")
            for k in range(NBLK):
                nc.vector.bn_aggr(mv[:, k, :], bst[:, k, :, :])
                nc.vector.tensor_scalar(
                    cc_sb[:, 2 * k:2 * k + 1], mv[:, k, 0:1], float(N), None, OP.mult)
                nc.vector.scalar_tensor_tensor(
                    cc_sb[:, 2 * k + 1:2 * k + 2], mv[:, k, 0:1],
                    mv[:, k, 0:1], mv[:, k, 1:2], OP.mult, OP.add)
                nc.vector.tensor_scalar(
                    cc_sb[:, 2 * k + 1:2 * k + 2], cc_sb[:, 2 * k + 1:2 * k + 2],
                    float(N), None, OP.mult)
            cc_in = dramp.tile([128, 4], F32, name=f"ccin_b{b}", tag=f"ccin{b}")
            cc_out = dramp.tile([128, 4], F32, name=f"ccout_b{b}", tag=f"ccout{b}",
                                addr_space="Shared")
            nc.sync.dma_start(out=cc_in, in_=cc_sb)
            nc.gpsimd.collective_compute(
                "AllReduce", OP.add, replica_groups=rg,
                ins=[cc_in[:, :]], outs=[cc_out[:, :]])
            return dict(xs=xs, cc_out=cc_out, cc_sb=cc_sb)

        def read_ccr(b, h):
            ccr = misc.tile([128, 4], F32, name=f"ccr_b{b}", tag=f"ccr{b}")
            nc.sync.dma_start(out=ccr, in_=h["cc_out"])
            return ccr

        def norm_store(b, ccr, xs, fence_src):
            # fence_src: chains the coefficient math (and so all normalize
            # work) behind the other branch's BN stats, so this branch's
            # normalize fills the other allreduce's latency window instead
            # of competing with compute.
            ccr3 = ccr.rearrange("p (k s) -> p k s", k=2)
            mean = misc.tile([128, 2], F32, name=f"mean_b{b}", tag=f"mean{b}")
            if fence_src is not None:
                tok = misc.tile([128, 2], F32, name=f"tok_b{b}", tag=f"tok{b}")
                nc.vector.tensor_scalar(tok, fence_src[:, 0:2], 0.0, None, OP.mult)
                nc.vector.scalar_tensor_tensor(
                    mean, ccr3[:, :, 0], 1.0 / BHW, tok, OP.mult, OP.add)
            else:
                nc.vector.tensor_scalar(mean, ccr3[:, :, 0], 1.0 / BHW, None,
                                        OP.mult)
            m2 = misc.tile([128, 2], F32, name=f"m2_b{b}", tag=f"m2{b}")
            nc.vector.tensor_mul(m2, mean, mean)
            var = misc.tile([128, 2], F32, name=f"var_b{b}", tag=f"var{b}")
            nc.vector.scalar_tensor_tensor(
                var, ccr3[:, :, 1], 1.0 / BHW, m2, OP.mult, OP.subtract)
            sd = misc.tile([128, 2], F32, name=f"sd_b{b}", tag=f"sd{b}")
            nc.scalar.activation(sd, var, AF.Sqrt, bias=epst)
            rs = misc.tile([128, 2], F32, name=f"rs_b{b}", tag=f"rs{b}")
            nc.vector.reciprocal(rs, sd)
            s2 = misc.tile([128, 2], F32, name=f"s2_b{b}", tag=f"s2{b}")
            nc.vector.tensor_mul(s2, rs, bp_sb[:, 0:2])
            ms = misc.tile([128, 2], F32, name=f"ms_b{b}", tag=f"ms{b}")
            nc.vector.tensor_mul(ms, mean, s2)
            t2 = misc.tile([128, 2], F32, name=f"t2_b{b}", tag=f"t2{b}")
            nc.vector.tensor_sub(t2, bp_sb[:, 2:4], ms)
            # normalize chunks alternate Vector/GpSimd, store via staging
            for k in range(NBLK):
                for jc in range(NSC):
                    sl = slice(jc * 1024, (jc + 1) * 1024)
                    st = stg.tile([128, 1024], F32, name=f"st_b{b}_{k}_{jc}",
                                  tag="stage")
                    eng = nc.vector if ((k * NSC + jc) % 2 == 0) else nc.gpsimd
                    eng.tensor_scalar(st, xs[k][:, sl], s2[:, k:k + 1],
                                      t2[:, k:k + 1], OP.mult, OP.add)
                    nc.sync.dma_start(out=out_d[b][k * 128:(k + 1) * 128, sl],
                                      in_=st)

        in0 = load_inputs(0)
        in1 = load_inputs(1)
        h0 = branch_compute(0, *in0)
        ccr0 = read_ccr(0, h0)
        h1 = branch_compute(1, *in1)
        norm_store(0, ccr0, h0["xs"], fence_src=h1["cc_sb"])
        ccr1 = read_ccr(1, h1)
        norm_store(1, ccr1, h1["xs"], fence_src=None)

    nc.finalize()
    return nc


def _get_nc():
    if "nc" not in _CACHE:
        _CACHE["nc"] = _build()
    return _CACHE["nc"]


def _pack_host(inputs):
    import ml_dtypes
    bf16 = ml_dtypes.bfloat16
    f32 = np.float32
    ws = []
    for b in ("1", "2"):
        for w in ("q", "k", "v"):
            ws.append(np.ascontiguousarray(
                np.asarray(inputs[f"w_{w}{b}"], dtype=f32).T))
    wt = np.concatenate(ws, axis=1).astype(bf16)  # [256, 1536]

    wps = []
    for b in ("1", "2"):
        wpT = np.ascontiguousarray(np.asarray(inputs[f"w_proj{b}"], dtype=f32).T)
        for p in range(2):
            wps.append(wpT[p * 128:(p + 1) * 128, :])
    wp = np.concatenate(wps, axis=1).astype(bf16)  # [128, 1024]

    g = np.asarray(inputs["gamma"], dtype=f32)
    be = np.asarray(inputs["beta"], dtype=f32)
    bp = np.stack([g[:128], g[128:], be[:128], be[128:]], axis=1)  # [128, 4]
    bv1 = np.asarray(inputs["b_v1"], dtype=f32)
    bv2 = np.asarray(inputs["b_v2"], dtype=f32)
    bh = np.stack([bv1[:128], bv1[128:], bv2[:128], bv2[128:]], axis=1)  # [128, 4]
    return (np.ascontiguousarray(wt), np.ascontiguousarray(wp),
            np.ascontiguousarray(bp), np.ascontiguousarray(bh))


def kernel(rgb_low, rgb_high, dsm_low, dsm_high,
           w_q1, b_q1, w_k1, b_k1, w_v1, b_v1,
           w_q2, b_q2, w_k2, b_k2, w_v2, b_v2,
           w_proj1, b_proj1, w_proj2, b_proj2, gamma, beta,
           _trace=False):
    from concourse.bass_utils import run_bass_kernel_spmd

    inputs = dict(w_q1=w_q1, w_k1=w_k1, w_v1=w_v1, w_proj1=w_proj1,
                  w_q2=w_q2, w_k2=w_k2, w_v2=w_v2, w_proj2=w_proj2,
                  b_v1=b_v1, b_v2=b_v2, gamma=gamma, beta=beta)
    f32 = np.float32
    rl = np.asarray(rgb_low, dtype=f32)
    rh = np.asarray(rgb_high, dtype=f32)
    dl = np.asarray(dsm_low, dtype=f32)
    dh = np.asarray(dsm_high, dtype=f32)
    B = rl.shape[0]
    assert B == NCORES, f"expected batch {NCORES}, got {B}"

    wt, wp, bp, bh = _pack_host(inputs)
    nc = _get_nc()

    in_maps = []
    for i in range(NCORES):
        in_maps.append({
            "xq1": np.ascontiguousarray(rl[i].reshape(C, N)),
            "xkv1": np.ascontiguousarray(dh[i].reshape(C, N)),
            "xq2": np.ascontiguousarray(rh[i].reshape(C, N)),
            "xkv2": np.ascontiguousarray(dl[i].reshape(C, N)),
            "wt": wt, "wp": wp, "bp": bp, "bh": bh,
        })

    res = run_bass_kernel_spmd(nc, in_maps, core_ids=list(range(NCORES)),
                               trace=_trace)
    out_low = np.stack([res.results[i]["out1"].reshape(C, 64, 64)
                        for i in range(NCORES)])
    out_high = np.stack([res.results[i]["out2"].reshape(C, 64, 64)
                         for i in range(NCORES)])
    if _trace:
        _CACHE["last_results"] = res
    return (out_low, out_high, np.asarray(dsm_low), np.asarray(dsm_high))


# revision 12
# speedup vs baseline: 1.1113x; 1.0103x over previous
# Trainium2 Bass kernel for nn_CrossFrequencyInteraction.
#
# Reference computation (per batch item, two symmetric branches):
#   q = Wq @ x_q;  k = Wk @ x_kv;  v = Wv @ x_kv          (1x1 convs, C=256)
#   out = softmax_n(q) used against ctx = softmax_n(k) @ v^T   (linear attention)
#   inter = Wp @ out;  x_q += inter
#   then training-mode BatchNorm over (B,H,W) on both updated rgb tensors.
#
# Sharding: data-parallel over batch (B=8 -> 1 item per core, 8 cores).
# BN statistics (per-channel sum/sumsq) are AllReduced across cores (2KB).
#
# Key algebraic restructurings (all exact):
#   - b_q, b_k shift softmax inputs by a per-row constant along the softmax
#     axis -> they cancel exactly; skipped.
#   - b_proj is a per-channel constant shift -> absorbed exactly by BN; skipped.
#   - b_v adds b_v[e] to ctx[d,e] (softmax_k sums to 1) -> folded into ctx.
#   - softmax normalizers (1/sum exp) for q and k are per-channel scales that
#     commute through the attention contraction -> folded into the tiny
#     M = Wp . blockdiag(ctx^T) matrix, so attention-out + proj become a
#     single [256,256] @ [256,4096] matmul per branch.
#   - kT/vT are produced directly in transposed layout by using x as the
#     stationary matmul operand (no explicit transposes anywhere); the
#     softmax-k denominators come from batched ones-row matmuls over paired
#     exp(k) tiles.
#
# Performance structure:
#   - ctx accumulation uses head-PAIR matmuls ([128,128] with garbage in the
#     cross-head blocks) into a [128,256] PSUM tile; evictions take only the
#     diagonal blocks into pre-zeroed block-diagonal SBUF tiles, so M^T is
#     2 matmuls per branch instead of 8 and ctx is 2 matmuls/tile not 4.
#   - x_q is loaded once, as bf16, via casting DMA (no Vector casts); the
#     residual x+inter is held in bf16 (error ~4e-4 << 2e-2 gate).
#   - PSUM->SBUF v^T evictions and the residual add run on GpSimd, BN stats
#     on Vector, exps on Scalar: no engine is oversubscribed.
#   - Branch-1's normalize+store is fenced to start only after branch-2's BN
#     stats are ready, so it fills the ~26us dead window of branch-2's
#     AllReduce instead of stealing Vector time from branch-2's compute.
#   - Input DMAs are chunked so the first KV matmul starts ~2MB into the
#     load, and the KV loop is software-pipelined (pkv(t+1) before ctx(t)).

import os
import numpy as np

C = 256
N = 4096
NBLK = 2          # channel blocks of 128
NT = 32           # n-tiles of 128 (for transposed convs)
NPAIR = NT // 2   # exp(k) tile pairs
NCH = 8           # n-chunks of 512 (for natural convs)
NH = 4            # heads
HD = 64           # head dim
NCORES = 8
BHW = 8 * 64 * 64  # BN reduction count
EPS = 1e-5
NLC = 4           # DMA load chunks per [128, N] input tile
NSC = 4           # store chunks of 1024 per channel block

_CACHE = {}


def _build():
    import concourse.bass as bass
    import concourse.bacc as bacc
    import concourse.tile as tile
    from concourse import mybir
    from contextlib import ExitStack

    F32 = mybir.dt.float32
    BF16 = mybir.dt.bfloat16
    OP = mybir.AluOpType
    AF = mybir.ActivationFunctionType
    AX = mybir.AxisListType

    nc = bacc.Bacc("TRN2", num_devices=NCORES)

    xq_d = [nc.dram_tensor(n_, [C, N], F32, kind="ExternalInput")
            for n_ in ("xq1", "xq2")]
    xkv_d = [nc.dram_tensor(n_, [C, N], F32, kind="ExternalInput")
             for n_ in ("xkv1", "xkv2")]
    # wt: [256, 6*256] = [Wk1^T|Wv1^T|Wq1^T|Wk2^T|Wv2^T|Wq2^T]  (need-order)
    wt_d = nc.dram_tensor("wt", [C, 6 * 256], BF16, kind="ExternalInput")
    # wp: [128, 4*256]; block (2b+p) = Wp_b^T[p*128:(p+1)*128, :]
    wp_d = nc.dram_tensor("wp", [128, 4 * 256], BF16, kind="ExternalInput")
    # bp: [128, 4] cols = (g_b0, g_b1, be_b0, be_b1)
    bp_d = nc.dram_tensor("bp", [128, 4], F32, kind="ExternalInput")
    # bh: [128, 4]; col (2b+p) = b_v[branch b][p*128:(p+1)*128]
    bh_d = nc.dram_tensor("bh", [128, 4], F32, kind="ExternalInput")
    out_d = [nc.dram_tensor(n_, [C, N], F32, kind="ExternalOutput")
             for n_ in ("out1", "out2")]

    with ExitStack() as ctx:
        tc = ctx.enter_context(tile.TileContext(nc))
        const = ctx.enter_context(tc.tile_pool(name="const", bufs=1))
        xqp = ctx.enter_context(tc.tile_pool(name="xqp", bufs=1))
        xkvp = ctx.enter_context(tc.tile_pool(name="xkvp", bufs=1))
        xsp = ctx.enter_context(tc.tile_pool(name="xsp", bufs=1))
        eqp = ctx.enter_context(tc.tile_pool(name="eqp", bufs=1))
        ekp = ctx.enter_context(tc.tile_pool(name="ekp", bufs=4))
        vtp = ctx.enter_context(tc.tile_pool(name="vtp", bufs=4))
        misc = ctx.enter_context(tc.tile_pool(name="misc", bufs=1))
        stg = ctx.enter_context(tc.tile_pool(name="stg", bufs=4))
        bigp = ctx.enter_context(tc.tile_pool(name="bigp", bufs=4, space="PSUM"))
        smallp = ctx.enter_context(tc.tile_pool(name="smallp", bufs=4, space="PSUM"))
        dramp = ctx.enter_context(tc.tile_pool(name="dramp", bufs=1, space="DRAM"))

        # ---- constants ----
        # wt loads are chunked in need-order (branch-0 KV block first) and
        # interleaved across the two channel-block tiles so the first KV
        # matmul can start after ~0.5MB of weight traffic.
        wt_sb = [const.tile([128, 6 * 256], BF16, name=f"wt{k}", tag=f"wt{k}")
                 for k in range(NBLK)]
        for c0, c1 in ((0, 512), (512, 768), (768, 1280), (1280, 1536)):
            for k in range(NBLK):
                nc.sync.dma_start(out=wt_sb[k][:, c0:c1],
                                  in_=wt_d[k * 128:(k + 1) * 128, c0:c1])
        wp_sb = const.tile([128, 4 * 256], BF16, name="wp", tag="wp")
        nc.sync.dma_start(out=wp_sb, in_=wp_d[:, :])
        bp_sb = const.tile([128, 4], F32, name="bp", tag="bp")
        nc.sync.dma_start(out=bp_sb, in_=bp_d[:, :])
        bh_sb = const.tile([128, 4], F32, name="bh", tag="bh")
        nc.sync.dma_start(out=bh_sb, in_=bh_d[:, :])
        ones_col = const.tile([128, 1], BF16, name="ones_col", tag="ones_col")
        nc.vector.memset(ones_col, 1.0)
        epst = const.tile([128, 1], F32, name="epst", tag="epst")
        nc.vector.memset(epst, EPS)
        # prime the scalar-engine activation tables (Exp, Sqrt) during DMA
        # warmup so the first real exp / the BN-coefficient sqrt don't eat
        # a ~1.3us ACT_TABLE_LOAD on the critical path.
        prim = const.tile([128, 1], F32, name="prim", tag="prim")
        nc.scalar.activation(prim, ones_col, AF.Exp)
        nc.scalar.activation(prim, ones_col, AF.Sqrt, bias=epst)
        # pre-zeroed block-diagonal ctx^T tiles (only diag blocks ever written)
        ctxq = []
        for i in range(4):
            t = const.tile([128, 128], BF16, name=f"ctxq{i}", tag=f"ctxq{i}")
            nc.vector.memset(t, 0.0)
            ctxq.append(t)

        expq = [eqp.tile([128, N], BF16, name=f"expq{k}", tag=f"expq{k}")
                for k in range(NBLK)]

        rg = [list(range(NCORES))]

        def load_inputs(b):
            # chunk loads along n, interleaved across the two channel blocks
            # (the first KV matmul needs chunk 0 of BOTH blocks).
            xkv = [xkvp.tile([128, N], BF16, name=f"xkv{k}_b{b}", tag=f"xkv{k}_b{b}")
                   for k in range(NBLK)]
            for jc in range(NLC):
                sl = slice(jc * (N // NLC), (jc + 1) * (N // NLC))
                for k in range(NBLK):
                    nc.gpsimd.dma_start(out=xkv[k][:, sl],
                                        in_=xkv_d[b][k * 128:(k + 1) * 128, sl])
            xqb = [xqp.tile([128, N], BF16, name=f"xqb{k}_b{b}", tag=f"xqb{k}_b{b}")
                   for k in range(NBLK)]
            for jc in range(NLC):
                sl = slice(jc * (N // NLC), (jc + 1) * (N // NLC))
                for k in range(NBLK):
                    nc.gpsimd.dma_start(out=xqb[k][:, sl],
                                        in_=xq_d[b][k * 128:(k + 1) * 128, sl])
            return xkv, xqb

        def branch_compute(b, xkv, xqb):
            wkv0 = b * 768
            wq0 = b * 768 + 512

            # ---- KV phase: kT|vT transposed convs + exp(k) + ctx/den accum
            pctx = smallp.tile([128, 256], F32, name=f"pctx_b{b}", tag="small")
            pden = smallp.tile([1, 512], F32, name=f"pden_b{b}", tag="small")
            ekw = [None] * NPAIR
            vts = [None] * NT

            def kv_front(t):
                pair, half = t // 2, t % 2
                if half == 0:
                    ekw[pair] = ekp.tile([128, 512], BF16,
                                         name=f"ekw_b{b}_{pair}", tag="ekw")
                pkv = bigp.tile([128, 512], F32, name=f"pkv_b{b}_{t}", tag="big")
                for k in range(NBLK):
                    nc.tensor.matmul(
                        pkv,
                        lhsT=xkv[k][:, t * 128:(t + 1) * 128],
                        rhs=wt_sb[k][:, wkv0:wkv0 + 512],
                        start=(k == 0), stop=(k == NBLK - 1))
                nc.scalar.activation(ekw[pair][:, half * 256:(half + 1) * 256],
                                     pkv[:, 0:256], AF.Exp)
                vt = vtp.tile([128, 256], BF16, name=f"vt_b{b}_{t}", tag="vt")
                nc.vector.tensor_copy(vt, pkv[:, 256:512])
                vts[t] = vt

            def kv_ctx(t):
                pair, half = t // 2, t % 2
                base = half * 256
                for p in range(2):
                    nc.tensor.matmul(
                        pctx[:, p * 128:(p + 1) * 128],
                        lhsT=vts[t][:, p * 128:(p + 1) * 128],
                        rhs=ekw[pair][:, base + p * 128:base + (p + 1) * 128],
                        start=(t == 0), stop=(t == NT - 1),
                        skip_group_check=True)
                if half == 1:
                    nc.tensor.matmul(
                        pden, lhsT=ones_col, rhs=ekw[pair][:, :],
                        start=(pair == 0), stop=(pair == NPAIR - 1),
                        skip_group_check=True)

            kv_front(0)
            for t in range(1, NT):
                kv_front(t)
                kv_ctx(t - 1)
            kv_ctx(NT - 1)

            # ---- softmax-k denominators: even+odd halves, transpose to cols
            drow_sb = misc.tile([1, 512], F32, name=f"drow_sb_b{b}", tag="drow_sb")
            nc.vector.tensor_copy(drow_sb, pden)
            drow = misc.tile([1, 256], F32, name=f"drow_b{b}", tag="drow")
            nc.vector.tensor_add(drow, drow_sb[0:1, 0:256], drow_sb[0:1, 256:512])
            denT = misc.tile([128, 2], F32, name=f"denT_b{b}", tag="denT")
            for k in range(NBLK):
                nc.scalar.dma_start(out=denT[:, k:k + 1],
                                    in_=drow[0:1, k * 128:(k + 1) * 128])

            # ---- ctx eviction (diag blocks only) with b_v fold ----
            for p in range(2):
                cq = ctxq[2 * b + p]
                col = 2 * b + p
                nc.vector.tensor_scalar(
                    cq[0:64, 0:64], pctx[0:64, p * 128:p * 128 + 64],
                    bh_sb[0:64, col:col + 1], None, OP.add)
                nc.vector.tensor_scalar(
                    cq[64:128, 64:128], pctx[64:128, p * 128 + 64:(p + 1) * 128],
                    bh_sb[64:128, col:col + 1], None, OP.add)

            # ---- M^T per head pair (tiny; PE runs these during Q phase) ----
            pmt = [smallp.tile([128, 256], F32, name=f"pmt_b{b}_{p}", tag="small")
                   for p in range(2)]
            for p in range(2):
                nc.tensor.matmul(
                    pmt[p], lhsT=ctxq[2 * b + p][:, :],
                    rhs=wp_sb[:, (2 * b + p) * 256:(2 * b + p + 1) * 256],
                    start=True, stop=True)

            # ---- Q phase: natural conv + exp with accumulated row sums ----
            sqp = misc.tile([128, NBLK, NCH], F32, name=f"sqp_b{b}", tag="sqp")
            sq2 = misc.tile([128, 2], F32, name=f"sq2_b{b}", tag="sq2")
            fde = misc.tile([128, 2], F32, name=f"fde_b{b}", tag="fde")
            fac = misc.tile([128, 2], F32, name=f"fac_b{b}", tag="fac")
            mt_sb = []
            for k in range(NBLK):
                for j in range(NCH):
                    pq = bigp.tile([128, 512], F32, name=f"pq_b{b}_{k}_{j}", tag="big")
                    for kk in range(NBLK):
                        nc.tensor.matmul(
                            pq,
                            lhsT=wt_sb[kk][:, wq0 + k * 128:wq0 + k * 128 + 128],
                            rhs=xqb[kk][:, j * 512:(j + 1) * 512],
                            start=(kk == 0), stop=(kk == NBLK - 1))
                    nc.scalar.activation(
                        expq[k][:, j * 512:(j + 1) * 512], pq, AF.Exp,
                        accum_out=sqp[:, k, j:j + 1])
                # normalization factor for this block: 1 / (denom_k * sum_q)
                nc.vector.reduce_sum(sq2[:, k:k + 1], sqp[:, k, :], axis=AX.X)
                nc.vector.tensor_mul(fde[:, k:k + 1], denT[:, k:k + 1],
                                     sq2[:, k:k + 1])
                nc.vector.reciprocal(fac[:, k:k + 1], fde[:, k:k + 1])
                mt = misc.tile([128, 256], BF16, name=f"mt_b{b}_{k}", tag=f"mt{k}")
                nc.vector.tensor_scalar(mt, pmt[k], fac[:, k:k + 1], None, OP.mult)
                mt_sb.append(mt)

            # ---- inter = M^T.T @ expq, residual into bf16 xs, BN stats ----
            xs = [xsp.tile([128, N], BF16, name=f"xs{k}_b{b}", tag=f"xs{k}_b{b}")
                  for k in range(NBLK)]
            bst = misc.tile([128, NBLK, NCH, 6], F32, name=f"bst_b{b}", tag="bst")
            for k in range(NBLK):
                for j in range(NCH):
                    pi = bigp.tile([128, 512], F32, name=f"pi_b{b}_{k}_{j}", tag="big")
                    for kk in range(NBLK):
                        nc.tensor.matmul(
                            pi,
                            lhsT=mt_sb[kk][:, k * 128:(k + 1) * 128],
                            rhs=expq[kk][:, j * 512:(j + 1) * 512],
                            start=(kk == 0), stop=(kk == NBLK - 1))
                    sl = slice(j * 512, (j + 1) * 512)
                    nc.vector.scalar_tensor_tensor(
                        xs[k][:, sl], pi, 1.0, xqb[k][:, sl], OP.mult, OP.add)
                    nc.vector.bn_stats(bst[:, k, j, :], xs[k][:, sl])

            # ---- local stats -> (sum, sumsq), launch allreduce ----
            cc_sb = misc.tile([128, 4], F32, name=f"cc_b{b}", tag=f"cc{b}")
            mv = misc.tile([128, NBLK, 2], F32, name=f"mv_b{b}", tag=f"mv{b}")
            for k in range(NBLK):
                nc.vector.bn_aggr(mv[:, k, :], bst[:, k, :, :])
                nc.vector.tensor_scalar(
                    cc_sb[:, 2 * k:2 * k + 1], mv[:, k, 0:1], float(N), None, OP.mult)
                nc.vector.scalar_tensor_tensor(
                    cc_sb[:, 2 * k + 1:2 * k + 2], mv[:, k, 0:1],
                    mv[:, k, 0:1], mv[:, k, 1:2], OP.mult, OP.add)
                nc.vector.tensor_scalar(
                    cc_sb[:, 2 * k + 1:2 * k + 2], cc_sb[:, 2 * k + 1:2 * k + 2],
                    float(N), None, OP.mult)
            cc_in = dramp.tile([128, 4], F32, name=f"ccin_b{b}", tag=f"ccin{b}")
            cc_out = dramp.tile([128, 4], F32, name=f"ccout_b{b}", tag=f"ccout{b}",
                                addr_space="Shared")
            nc.sync.dma_start(out=cc_in, in_=cc_sb)
            nc.gpsimd.collective_compute(
                "AllReduce", OP.add, replica_groups=rg,
                ins=[cc_in[:, :]], outs=[cc_out[:, :]])
            return dict(xs=xs, cc_out=cc_out, cc_sb=cc_sb)

        def read_ccr(b, h):
            ccr = misc.tile([128, 4], F32, name=f"ccr_b{b}", tag=f"ccr{b}")
            nc.sync.dma_start(out=ccr, in_=h["cc_out"])
            return ccr

        def norm_store(b, ccr, xs, fence_src):
            # fence_src: chains the coefficient math (and so all normalize
            # work) behind the other branch's BN stats, so this branch's
            # normalize fills the other allreduce's latency window instead
            # of competing with compute.
            ccr3 = ccr.rearrange("p (k s) -> p k s", k=2)
            mean = misc.tile([128, 2], F32, name=f"mean_b{b}", tag=f"mean{b}")
            if fence_src is not None:
                tok = misc.tile([128, 2], F32, name=f"tok_b{b}", tag=f"tok{b}")
                nc.vector.tensor_scalar(tok, fence_src[:, 0:2], 0.0, None, OP.mult)
                nc.vector.scalar_tensor_tensor(
                    mean, ccr3[:, :, 0], 1.0 / BHW, tok, OP.mult, OP.add)
            else:
                nc.vector.tensor_scalar(mean, ccr3[:, :, 0], 1.0 / BHW, None,
                                        OP.mult)
            m2 = misc.tile([128, 2], F32, name=f"m2_b{b}", tag=f"m2{b}")
            nc.vector.tensor_mul(m2, mean, mean)
            var = misc.tile([128, 2], F32, name=f"var_b{b}", tag=f"var{b}")
            nc.vector.scalar_tensor_tensor(
                var, ccr3[:, :, 1], 1.0 / BHW, m2, OP.mult, OP.subtract)
            sd = misc.tile([128, 2], F32, name=f"sd_b{b}", tag=f"sd{b}")
            nc.scalar.activation(sd, var, AF.Sqrt, bias=epst)
            rs = misc.tile([128, 2], F32, name=f"rs_b{b}", tag=f"rs{b}")
            nc.vector.reciprocal(rs, sd)
            s2 = misc.tile([128, 2], F32, name=f"s2_b{b}", tag=f"s2{b}")
            nc.vector.tensor_mul(s2, rs, bp_sb[:, 0:2])
            ms = misc.tile([128, 2], F32, name=f"ms_b{b}", tag=f"ms{b}")
            nc.vector.tensor_mul(ms, mean, s2)
            t2 = misc.tile([128, 2], F32, name=f"t2_b{b}", tag=f"t2{b}")
            nc.vector.tensor_sub(t2, bp_sb[:, 2:4], ms)
            # Normalize + store. For the branch hidden under the other's
            # allreduce (fence_src set) the window is wide: alternate
            # Vector/GpSimd. For the LAST branch the chunks are on the
            # critical path: keep them all on Vector (GpSimd is ~2x slower)
            # with finer chunks, and alternate store pushes between the
            # sync and scalar DMA queues to halve descriptor serialization.
            csz = 1024 if fence_src is not None else 512
            nch_st = N // csz
            i = 0
            for k in range(NBLK):
                for jc in range(nch_st):
                    sl = slice(jc * csz, (jc + 1) * csz)
                    st = stg.tile([128, csz], F32, name=f"st_b{b}_{k}_{jc}",
                                  tag="stage" if csz == 1024 else "stage5")
                    if fence_src is not None:
                        eng = nc.vector if (i % 2 == 0) else nc.gpsimd
                    else:
                        eng = nc.vector
                    eng.tensor_scalar(st, xs[k][:, sl], s2[:, k:k + 1],
                                      t2[:, k:k + 1], OP.mult, OP.add)
                    q = nc.sync if (i % 2 == 0) else nc.scalar
                    q.dma_start(out=out_d[b][k * 128:(k + 1) * 128, sl], in_=st)
                    i += 1

        in0 = load_inputs(0)
        in1 = load_inputs(1)
        h0 = branch_compute(0, *in0)
        ccr0 = read_ccr(0, h0)
        h1 = branch_compute(1, *in1)
        norm_store(0, ccr0, h0["xs"], fence_src=h1["cc_sb"])
        ccr1 = read_ccr(1, h1)
        norm_store(1, ccr1, h1["xs"], fence_src=None)

    nc.finalize()
    return nc


def _get_nc():
    if "nc" not in _CACHE:
        _CACHE["nc"] = _build()
    return _CACHE["nc"]


def _pack_host(inputs):
    import ml_dtypes
    bf16 = ml_dtypes.bfloat16
    f32 = np.float32
    ws = []
    for b in ("1", "2"):
        for w in ("k", "v", "q"):
            ws.append(np.ascontiguousarray(
                np.asarray(inputs[f"w_{w}{b}"], dtype=f32).T))
    wt = np.concatenate(ws, axis=1).astype(bf16)  # [256, 1536]

    wps = []
    for b in ("1", "2"):
        wpT = np.ascontiguousarray(np.asarray(inputs[f"w_proj{b}"], dtype=f32).T)
        for p in range(2):
            wps.append(wpT[p * 128:(p + 1) * 128, :])
    wp = np.concatenate(wps, axis=1).astype(bf16)  # [128, 1024]

    g = np.asarray(inputs["gamma"], dtype=f32)
    be = np.asarray(inputs["beta"], dtype=f32)
    bp = np.stack([g[:128], g[128:], be[:128], be[128:]], axis=1)  # [128, 4]
    bv1 = np.asarray(inputs["b_v1"], dtype=f32)
    bv2 = np.asarray(inputs["b_v2"], dtype=f32)
    bh = np.stack([bv1[:128], bv1[128:], bv2[:128], bv2[128:]], axis=1)  # [128, 4]
    return (np.ascontiguousarray(wt), np.ascontiguousarray(wp),
            np.ascontiguousarray(bp), np.ascontiguousarray(bh))


def kernel(rgb_low, rgb_high, dsm_low, dsm_high,
           w_q1, b_q1, w_k1, b_k1, w_v1, b_v1,
           w_q2, b_q2, w_k2, b_k2, w_v2, b_v2,
           w_proj1, b_proj1, w_proj2, b_proj2, gamma, beta,
           _trace=False):
    from concourse.bass_utils import run_bass_kernel_spmd

    inputs = dict(w_q1=w_q1, w_k1=w_k1, w_v1=w_v1, w_proj1=w_proj1,
                  w_q2=w_q2, w_k2=w_k2, w_v2=w_v2, w_proj2=w_proj2,
                  b_v1=b_v1, b_v2=b_v2, gamma=gamma, beta=beta)
    f32 = np.float32
    rl = np.asarray(rgb_low, dtype=f32)
    rh = np.asarray(rgb_high, dtype=f32)
    dl = np.asarray(dsm_low, dtype=f32)
    dh = np.asarray(dsm_high, dtype=f32)
    B = rl.shape[0]
    assert B == NCORES, f"expected batch {NCORES}, got {B}"

    wt, wp, bp, bh = _pack_host(inputs)
    nc = _get_nc()

    in_maps = []
    for i in range(NCORES):
        in_maps.append({
            "xq1": np.ascontiguousarray(rl[i].reshape(C, N)),
            "xkv1": np.ascontiguousarray(dh[i].reshape(C, N)),
            "xq2": np.ascontiguousarray(rh[i].reshape(C, N)),
            "xkv2": np.ascontiguousarray(dl[i].reshape(C, N)),
            "wt": wt, "wp": wp, "bp": bp, "bh": bh,
        })

    res = run_bass_kernel_spmd(nc, in_maps, core_ids=list(range(NCORES)),
                               trace=_trace)
    out_low = np.stack([res.results[i]["out1"].reshape(C, 64, 64)
                        for i in range(NCORES)])
    out_high = np.stack([res.results[i]["out2"].reshape(C, 64, 64)
                         for i in range(NCORES)])
    if _trace:
        _CACHE["last_results"] = res
    return (out_low, out_high, np.asarray(dsm_low), np.asarray(dsm_high))


# revision 21
# speedup vs baseline: 1.1662x; 1.0494x over previous
# Trainium2 Bass kernel for nn_CrossFrequencyInteraction.
#
# Reference computation (per batch item, two symmetric branches):
#   q = Wq @ x_q;  k = Wk @ x_kv;  v = Wv @ x_kv          (1x1 convs, C=256)
#   out = softmax_n(q) used against ctx = softmax_n(k) @ v^T   (linear attention)
#   inter = Wp @ out;  x_q += inter
#   then training-mode BatchNorm over (B,H,W) on both updated rgb tensors.
#
# Sharding: data-parallel over batch (B=8 -> 1 item per core, 8 cores).
# BN statistics (per-channel sum/sumsq) are AllReduced across cores (2KB).
#
# Key algebraic restructurings (all exact):
#   - b_q, b_k shift softmax inputs by a per-row constant along the softmax
#     axis -> they cancel exactly; skipped.
#   - b_proj is a per-channel constant shift -> absorbed exactly by BN; skipped.
#   - b_v adds b_v[e] to ctx[d,e] (softmax_k sums to 1) -> folded into ctx.
#   - softmax normalizers (1/sum exp) for q and k are per-channel scales that
#     commute through the attention contraction -> folded into the tiny
#     M = Wp . blockdiag(ctx^T) matrix, so attention-out + proj become a
#     single [256,256] @ [256,4096] matmul per branch.
#   - kT/vT are produced directly in transposed layout by using x as the
#     stationary matmul operand (no explicit transposes anywhere); the
#     softmax-k denominators come from batched ones-row matmuls over paired
#     exp(k) tiles.
#
# Performance structure:
#   - ctx accumulation uses head-PAIR matmuls ([128,128] with garbage in the
#     cross-head blocks) into a [128,256] PSUM tile; evictions take only the
#     diagonal blocks into pre-zeroed block-diagonal SBUF tiles, so M^T is
#     2 matmuls per branch instead of 8 and ctx is 2 matmuls/tile not 4.
#   - x_q is loaded once, as bf16, via casting DMA (no Vector casts); the
#     residual x+inter is held in bf16 (error ~4e-4 << 2e-2 gate).
#   - PSUM->SBUF v^T evictions and the residual add run on GpSimd, BN stats
#     on Vector, exps on Scalar: no engine is oversubscribed.
#   - Branch-1's normalize+store is fenced to start only after branch-2's BN
#     stats are ready, so it fills the ~26us dead window of branch-2's
#     AllReduce instead of stealing Vector time from branch-2's compute.
#   - Input DMAs are chunked so the first KV matmul starts ~2MB into the
#     load, and the KV loop is software-pipelined (pkv(t+1) before ctx(t)).

import os
import numpy as np

C = 256
N = 4096
NBLK = 2          # channel blocks of 128
NT = 32           # n-tiles of 128 (for transposed convs)
NPAIR = NT // 2   # exp(k) tile pairs
NCH = 8           # n-chunks of 512 (for natural convs)
NH = 4            # heads
HD = 64           # head dim
NCORES = 8
BHW = 8 * 64 * 64  # BN reduction count
EPS = 1e-5
NLC = 4           # DMA load chunks per [128, N] input tile
NSC = 4           # store chunks of 1024 per channel block

_CACHE = {}


def _build():
    import concourse.bass as bass
    import concourse.bacc as bacc
    import concourse.tile as tile
    from concourse import mybir
    from contextlib import ExitStack

    F32 = mybir.dt.float32
    BF16 = mybir.dt.bfloat16
    OP = mybir.AluOpType
    AF = mybir.ActivationFunctionType
    AX = mybir.AxisListType

    nc = bacc.Bacc("TRN2", num_devices=NCORES)

    xq_d = [nc.dram_tensor(n_, [C, N], F32, kind="ExternalInput")
            for n_ in ("xq1", "xq2")]
    xkv_d = [nc.dram_tensor(n_, [C, N], F32, kind="ExternalInput")
             for n_ in ("xkv1", "xkv2")]
    # wt: [256, 6*256] = [Wk1^T|Wv1^T|Wq1^T|Wk2^T|Wv2^T|Wq2^T]  (need-order)
    wt_d = nc.dram_tensor("wt", [C, 6 * 256], BF16, kind="ExternalInput")
    # wp: [128, 4*256]; block (2b+p) = Wp_b^T[p*128:(p+1)*128, :]
    wp_d = nc.dram_tensor("wp", [128, 4 * 256], BF16, kind="ExternalInput")
    # bp: [128, 4] cols = (g_b0, g_b1, be_b0, be_b1)
    bp_d = nc.dram_tensor("bp", [128, 4], F32, kind="ExternalInput")
    # bh: [128, 4]; col (2b+p) = b_v[branch b][p*128:(p+1)*128]
    bh_d = nc.dram_tensor("bh", [128, 4], F32, kind="ExternalInput")
    out_d = [nc.dram_tensor(n_, [C, N], F32, kind="ExternalOutput")
             for n_ in ("out1", "out2")]

    with ExitStack() as ctx:
        tc = ctx.enter_context(tile.TileContext(nc))
        const = ctx.enter_context(tc.tile_pool(name="const", bufs=1))
        xqp = ctx.enter_context(tc.tile_pool(name="xqp", bufs=1))
        xkvp = ctx.enter_context(tc.tile_pool(name="xkvp", bufs=1))
        xsp = ctx.enter_context(tc.tile_pool(name="xsp", bufs=1))
        eqp = ctx.enter_context(tc.tile_pool(name="eqp", bufs=1))
        ekp = ctx.enter_context(tc.tile_pool(name="ekp", bufs=4))
        vtp = ctx.enter_context(tc.tile_pool(name="vtp", bufs=4))
        misc = ctx.enter_context(tc.tile_pool(name="misc", bufs=1))
        stg = ctx.enter_context(tc.tile_pool(name="stg", bufs=8))
        bigp = ctx.enter_context(tc.tile_pool(name="bigp", bufs=4, space="PSUM"))
        smallp = ctx.enter_context(tc.tile_pool(name="smallp", bufs=4, space="PSUM"))
        dramp = ctx.enter_context(tc.tile_pool(name="dramp", bufs=1, space="DRAM"))

        # ---- constants ----
        # wt loads are chunked in need-order (branch-0 KV block first) and
        # interleaved across the two channel-block tiles so the first KV
        # matmul can start after ~0.5MB of weight traffic.
        wt_sb = [const.tile([128, 6 * 256], BF16, name=f"wt{k}", tag=f"wt{k}")
                 for k in range(NBLK)]
        for c0, c1 in ((0, 512), (512, 768), (768, 1280), (1280, 1536)):
            for k in range(NBLK):
                nc.sync.dma_start(out=wt_sb[k][:, c0:c1],
                                  in_=wt_d[k * 128:(k + 1) * 128, c0:c1])
        wp_sb = const.tile([128, 4 * 256], BF16, name="wp", tag="wp")
        nc.sync.dma_start(out=wp_sb, in_=wp_d[:, :])
        bp_sb = const.tile([128, 4], F32, name="bp", tag="bp")
        nc.sync.dma_start(out=bp_sb, in_=bp_d[:, :])
        bh_sb = const.tile([128, 4], F32, name="bh", tag="bh")
        nc.sync.dma_start(out=bh_sb, in_=bh_d[:, :])
        ones_col = const.tile([128, 1], BF16, name="ones_col", tag="ones_col")
        nc.vector.memset(ones_col, 1.0)
        epst = const.tile([128, 1], F32, name="epst", tag="epst")
        nc.vector.memset(epst, EPS)
        # prime the scalar-engine activation tables (Exp, Sqrt) during DMA
        # warmup so the first real exp / the BN-coefficient sqrt don't eat
        # a ~1.3us ACT_TABLE_LOAD on the critical path.
        prim = const.tile([128, 1], F32, name="prim", tag="prim")
        nc.scalar.activation(prim, ones_col, AF.Exp)
        nc.scalar.activation(prim, ones_col, AF.Sqrt, bias=epst)
        # pre-zeroed block-diagonal ctx^T tiles (only diag blocks ever written)
        ctxq = []
        for i in range(4):
            t = const.tile([128, 128], BF16, name=f"ctxq{i}", tag=f"ctxq{i}")
            nc.vector.memset(t, 0.0)
            ctxq.append(t)

        expq = [eqp.tile([128, N], BF16, name=f"expq{k}", tag=f"expq{k}")
                for k in range(NBLK)]

        rg = [list(range(NCORES))]

        def load_inputs(b):
            # chunk loads along n, interleaved across the two channel blocks
            # (the first KV matmul needs chunk 0 of BOTH blocks).
            xkv = [xkvp.tile([128, N], BF16, name=f"xkv{k}_b{b}", tag=f"xkv{k}_b{b}")
                   for k in range(NBLK)]
            for jc in range(NLC):
                sl = slice(jc * (N // NLC), (jc + 1) * (N // NLC))
                for k in range(NBLK):
                    nc.gpsimd.dma_start(out=xkv[k][:, sl],
                                        in_=xkv_d[b][k * 128:(k + 1) * 128, sl])
            xqb = [xqp.tile([128, N], BF16, name=f"xqb{k}_b{b}", tag=f"xqb{k}_b{b}")
                   for k in range(NBLK)]
            for jc in range(NLC):
                sl = slice(jc * (N // NLC), (jc + 1) * (N // NLC))
                for k in range(NBLK):
                    nc.gpsimd.dma_start(out=xqb[k][:, sl],
                                        in_=xq_d[b][k * 128:(k + 1) * 128, sl])
            return xkv, xqb

        def branch_compute(b, xkv, xqb):
            wkv0 = b * 768
            wq0 = b * 768 + 512

            # ---- KV phase: kT|vT transposed convs + exp(k) + ctx/den accum
            pctx = smallp.tile([128, 256], F32, name=f"pctx_b{b}", tag="small")
            pden = smallp.tile([1, 512], F32, name=f"pden_b{b}", tag="small")
            ekw = [None] * NPAIR
            vts = [None] * NT

            def kv_front(t):
                pair, half = t // 2, t % 2
                if half == 0:
                    ekw[pair] = ekp.tile([128, 512], BF16,
                                         name=f"ekw_b{b}_{pair}", tag="ekw")
                pkv = bigp.tile([128, 512], F32, name=f"pkv_b{b}_{t}", tag="big")
                for k in range(NBLK):
                    nc.tensor.matmul(
                        pkv,
                        lhsT=xkv[k][:, t * 128:(t + 1) * 128],
                        rhs=wt_sb[k][:, wkv0:wkv0 + 512],
                        start=(k == 0), stop=(k == NBLK - 1))
                nc.scalar.activation(ekw[pair][:, half * 256:(half + 1) * 256],
                                     pkv[:, 0:256], AF.Exp)
                vt = vtp.tile([128, 256], BF16, name=f"vt_b{b}_{t}", tag="vt")
                nc.vector.tensor_copy(vt, pkv[:, 256:512])
                vts[t] = vt

            def kv_ctx(t):
                pair, half = t // 2, t % 2
                base = half * 256
                for p in range(2):
                    nc.tensor.matmul(
                        pctx[:, p * 128:(p + 1) * 128],
                        lhsT=vts[t][:, p * 128:(p + 1) * 128],
                        rhs=ekw[pair][:, base + p * 128:base + (p + 1) * 128],
                        start=(t == 0), stop=(t == NT - 1),
                        skip_group_check=True)
                if half == 1:
                    nc.tensor.matmul(
                        pden, lhsT=ones_col, rhs=ekw[pair][:, :],
                        start=(pair == 0), stop=(pair == NPAIR - 1),
                        skip_group_check=True)

            kv_front(0)
            for t in range(1, NT):
                kv_front(t)
                kv_ctx(t - 1)
            kv_ctx(NT - 1)

            # ---- softmax-k denominators: even+odd halves, transpose to cols
            drow_sb = misc.tile([1, 512], F32, name=f"drow_sb_b{b}", tag="drow_sb")
            nc.vector.tensor_copy(drow_sb, pden)
            drow = misc.tile([1, 256], F32, name=f"drow_b{b}", tag="drow")
            nc.vector.tensor_add(drow, drow_sb[0:1, 0:256], drow_sb[0:1, 256:512])
            denT = misc.tile([128, 2], F32, name=f"denT_b{b}", tag="denT")
            for k in range(NBLK):
                nc.scalar.dma_start(out=denT[:, k:k + 1],
                                    in_=drow[0:1, k * 128:(k + 1) * 128])

            # ---- ctx eviction (diag blocks only) with b_v fold ----
            for p in range(2):
                cq = ctxq[2 * b + p]
                col = 2 * b + p
                nc.vector.tensor_scalar(
                    cq[0:64, 0:64], pctx[0:64, p * 128:p * 128 + 64],
                    bh_sb[0:64, col:col + 1], None, OP.add)
                nc.vector.tensor_scalar(
                    cq[64:128, 64:128], pctx[64:128, p * 128 + 64:(p + 1) * 128],
                    bh_sb[64:128, col:col + 1], None, OP.add)

            # ---- M^T per head pair (tiny; PE runs these during Q phase) ----
            pmt = [smallp.tile([128, 256], F32, name=f"pmt_b{b}_{p}", tag="small")
                   for p in range(2)]
            for p in range(2):
                nc.tensor.matmul(
                    pmt[p], lhsT=ctxq[2 * b + p][:, :],
                    rhs=wp_sb[:, (2 * b + p) * 256:(2 * b + p + 1) * 256],
                    start=True, stop=True)

            # ---- Q phase: natural conv + exp with accumulated row sums ----
            sqp = misc.tile([128, NBLK, NCH], F32, name=f"sqp_b{b}", tag="sqp")
            sq2 = misc.tile([128, 2], F32, name=f"sq2_b{b}", tag="sq2")
            fde = misc.tile([128, 2], F32, name=f"fde_b{b}", tag="fde")
            fac = misc.tile([128, 2], F32, name=f"fac_b{b}", tag="fac")
            mt_sb = []
            for k in range(NBLK):
                for j in range(NCH):
                    pq = bigp.tile([128, 512], F32, name=f"pq_b{b}_{k}_{j}", tag="big")
                    for kk in range(NBLK):
                        nc.tensor.matmul(
                            pq,
                            lhsT=wt_sb[kk][:, wq0 + k * 128:wq0 + k * 128 + 128],
                            rhs=xqb[kk][:, j * 512:(j + 1) * 512],
                            start=(kk == 0), stop=(kk == NBLK - 1))
                    nc.scalar.activation(
                        expq[k][:, j * 512:(j + 1) * 512], pq, AF.Exp,
                        accum_out=sqp[:, k, j:j + 1])
                # normalization factor for this block: 1 / (denom_k * sum_q)
                nc.vector.reduce_sum(sq2[:, k:k + 1], sqp[:, k, :], axis=AX.X)
                nc.vector.tensor_mul(fde[:, k:k + 1], denT[:, k:k + 1],
                                     sq2[:, k:k + 1])
                nc.vector.reciprocal(fac[:, k:k + 1], fde[:, k:k + 1])
                mt = misc.tile([128, 256], BF16, name=f"mt_b{b}_{k}", tag=f"mt{k}")
                nc.vector.tensor_scalar(mt, pmt[k], fac[:, k:k + 1], None, OP.mult)
                mt_sb.append(mt)

            # ---- inter = M^T.T @ expq, residual into bf16 xs, BN stats ----
            xs = [xsp.tile([128, N], BF16, name=f"xs{k}_b{b}", tag=f"xs{k}_b{b}")
                  for k in range(NBLK)]
            bst = misc.tile([128, NBLK, NCH, 6], F32, name=f"bst_b{b}", tag="bst")
            cc_sb = misc.tile([128, 4], F32, name=f"cc_b{b}", tag=f"cc{b}")
            mv = misc.tile([128, NBLK, 2], F32, name=f"mv_b{b}", tag=f"mv{b}")
            cc_in = dramp.tile([128, 4], F32, name=f"ccin_b{b}", tag=f"ccin{b}")
            for k in range(NBLK):
                for j in range(NCH):
                    pi = bigp.tile([128, 512], F32, name=f"pi_b{b}_{k}_{j}", tag="big")
                    for kk in range(NBLK):
                        nc.tensor.matmul(
                            pi,
                            lhsT=mt_sb[kk][:, k * 128:(k + 1) * 128],
                            rhs=expq[kk][:, j * 512:(j + 1) * 512],
                            start=(kk == 0), stop=(kk == NBLK - 1))
                    sl = slice(j * 512, (j + 1) * 512)
                    nc.vector.scalar_tensor_tensor(
                        xs[k][:, sl], pi, 1.0, xqb[k][:, sl], OP.mult, OP.add)
                    nc.vector.bn_stats(bst[:, k, j, :], xs[k][:, sl])
                # aggregate + pack + upload this block's (sum, sumsq) now;
                # k=0's share runs under k=1's compute, off the critical path
                nc.vector.bn_aggr(mv[:, k, :], bst[:, k, :, :])
                nc.vector.tensor_scalar(
                    cc_sb[:, 2 * k:2 * k + 1], mv[:, k, 0:1], float(N), None, OP.mult)
                nc.vector.scalar_tensor_tensor(
                    cc_sb[:, 2 * k + 1:2 * k + 2], mv[:, k, 0:1],
                    mv[:, k, 0:1], mv[:, k, 1:2], OP.mult, OP.add)
                nc.vector.tensor_scalar(
                    cc_sb[:, 2 * k + 1:2 * k + 2], cc_sb[:, 2 * k + 1:2 * k + 2],
                    float(N), None, OP.mult)
                nc.sync.dma_start(out=cc_in[:, 2 * k:2 * k + 2],
                                  in_=cc_sb[:, 2 * k:2 * k + 2])

            # ---- launch AllGather of packed local stats ----
            # (AllGather of [128,4] + local 8-way sum is ~2x lower latency
            # than AllReduce for this latency-bound 2KB exchange.)
            cc_out = dramp.tile([NCORES * 128, 4], F32, name=f"ccout_b{b}",
                                tag=f"ccout{b}", addr_space="Shared")
            nc.gpsimd.collective_compute(
                "AllGather", OP.bypass, replica_groups=rg,
                ins=[cc_in[:, :]], outs=[cc_out[:, :]])
            return dict(xs=xs, cc_out=cc_out, cc_sb=cc_sb)

        def read_ccr(b, h):
            # read back the gathered [128, 8*4] stats and tree-sum the 8
            # per-core contributions into [128, 4]
            gat = misc.tile([128, NCORES, 4], F32, name=f"gat_b{b}", tag=f"gat{b}")
            # DRAM AllGather concatenates rank buffers flat (rank-major):
            # strided view puts rank r's [128,4] block at gat[:, r, :]
            nc.sync.dma_start(
                out=gat, in_=h["cc_out"].rearrange("(r p) s -> p r s", r=NCORES))
            h1 = misc.tile([128, 4, 4], F32, name=f"h1_b{b}", tag=f"h1{b}")
            nc.vector.tensor_add(h1, gat[:, 0:4, :], gat[:, 4:8, :])
            h2 = misc.tile([128, 2, 4], F32, name=f"h2_b{b}", tag=f"h2{b}")
            nc.vector.tensor_add(h2, h1[:, 0:2, :], h1[:, 2:4, :])
            ccr = misc.tile([128, 4], F32, name=f"ccr_b{b}", tag=f"ccr{b}")
            nc.vector.tensor_add(ccr, h2[:, 0, :], h2[:, 1, :])
            return ccr

        def norm_store(b, ccr, xs, fence_src):
            # fence_src: chains the coefficient math (and so all normalize
            # work) behind the other branch's BN stats, so this branch's
            # normalize fills the other allreduce's latency window instead
            # of competing with compute.
            ccr3 = ccr.rearrange("p (k s) -> p k s", k=2)
            mean = misc.tile([128, 2], F32, name=f"mean_b{b}", tag=f"mean{b}")
            if fence_src is not None:
                tok = misc.tile([128, 2], F32, name=f"tok_b{b}", tag=f"tok{b}")
                nc.vector.tensor_scalar(tok, fence_src[:, 0:2], 0.0, None, OP.mult)
                nc.vector.scalar_tensor_tensor(
                    mean, ccr3[:, :, 0], 1.0 / BHW, tok, OP.mult, OP.add)
            else:
                nc.vector.tensor_scalar(mean, ccr3[:, :, 0], 1.0 / BHW, None,
                                        OP.mult)
            m2 = misc.tile([128, 2], F32, name=f"m2_b{b}", tag=f"m2{b}")
            nc.vector.tensor_mul(m2, mean, mean)
            var = misc.tile([128, 2], F32, name=f"var_b{b}", tag=f"var{b}")
            nc.vector.scalar_tensor_tensor(
                var, ccr3[:, :, 1], 1.0 / BHW, m2, OP.mult, OP.subtract)
            sd = misc.tile([128, 2], F32, name=f"sd_b{b}", tag=f"sd{b}")
            nc.scalar.activation(sd, var, AF.Sqrt, bias=epst)
            rs = misc.tile([128, 2], F32, name=f"rs_b{b}", tag=f"rs{b}")
            nc.vector.reciprocal(rs, sd)
            s2 = misc.tile([128, 2], F32, name=f"s2_b{b}", tag=f"s2{b}")
            nc.vector.tensor_mul(s2, rs, bp_sb[:, 0:2])
            ms = misc.tile([128, 2], F32, name=f"ms_b{b}", tag=f"ms{b}")
            nc.vector.tensor_mul(ms, mean, s2)
            t2 = misc.tile([128, 2], F32, name=f"t2_b{b}", tag=f"t2{b}")
            nc.vector.tensor_sub(t2, bp_sb[:, 2:4], ms)
            # Normalize + store. For the branch hidden under the other's
            # allreduce (fence_src set) the window is wide: alternate
            # Vector/GpSimd. For the LAST branch the chunks are on the
            # critical path: keep them all on Vector (GpSimd is ~2x slower)
            # with finer chunks, and alternate store pushes between the
            # sync and scalar DMA queues to halve descriptor serialization.
            csz = 1024
            nch_st = N // csz
            i = 0
            for k in range(NBLK):
                for jc in range(nch_st):
                    sl = slice(jc * csz, (jc + 1) * csz)
                    st = stg.tile([128, csz], F32, name=f"st_b{b}_{k}_{jc}",
                                  tag="stage")
                    if fence_src is not None:
                        eng = nc.vector if (i % 2 == 0) else nc.gpsimd
                    else:
                        eng = nc.vector
                    eng.tensor_scalar(st, xs[k][:, sl], s2[:, k:k + 1],
                                      t2[:, k:k + 1], OP.mult, OP.add)
                    q = nc.sync if (i % 2 == 0) else nc.scalar
                    q.dma_start(out=out_d[b][k * 128:(k + 1) * 128, sl], in_=st)
                    i += 1

        in0 = load_inputs(0)
        in1 = load_inputs(1)
        h0 = branch_compute(0, *in0)
        ccr0 = read_ccr(0, h0)
        h1 = branch_compute(1, *in1)
        norm_store(0, ccr0, h0["xs"], fence_src=h1["cc_sb"])
        ccr1 = read_ccr(1, h1)
        norm_store(1, ccr1, h1["xs"], fence_src=None)

    nc.finalize()
    return nc


def _get_nc():
    if "nc" not in _CACHE:
        _CACHE["nc"] = _build()
    return _CACHE["nc"]


def _pack_host(inputs):
    import ml_dtypes
    bf16 = ml_dtypes.bfloat16
    f32 = np.float32
    ws = []
    for b in ("1", "2"):
        for w in ("k", "v", "q"):
            ws.append(np.ascontiguousarray(
                np.asarray(inputs[f"w_{w}{b}"], dtype=f32).T))
    wt = np.concatenate(ws, axis=1).astype(bf16)  # [256, 1536]

    wps = []
    for b in ("1", "2"):
        wpT = np.ascontiguousarray(np.asarray(inputs[f"w_proj{b}"], dtype=f32).T)
        for p in range(2):
            wps.append(wpT[p * 128:(p + 1) * 128, :])
    wp = np.concatenate(wps, axis=1).astype(bf16)  # [128, 1024]

    g = np.asarray(inputs["gamma"], dtype=f32)
    be = np.asarray(inputs["beta"], dtype=f32)
    bp = np.stack([g[:128], g[128:], be[:128], be[128:]], axis=1)  # [128, 4]
    bv1 = np.asarray(inputs["b_v1"], dtype=f32)
    bv2 = np.asarray(inputs["b_v2"], dtype=f32)
    bh = np.stack([bv1[:128], bv1[128:], bv2[:128], bv2[128:]], axis=1)  # [128, 4]
    return (np.ascontiguousarray(wt), np.ascontiguousarray(wp),
            np.ascontiguousarray(bp), np.ascontiguousarray(bh))


def kernel(rgb_low, rgb_high, dsm_low, dsm_high,
           w_q1, b_q1, w_k1, b_k1, w_v1, b_v1,
           w_q2, b_q2, w_k2, b_k2, w_v2, b_v2,
           w_proj1, b_proj1, w_proj2, b_proj2, gamma, beta,
           _trace=False):
    from concourse.bass_utils import run_bass_kernel_spmd

    inputs = dict(w_q1=w_q1, w_k1=w_k1, w_v1=w_v1, w_proj1=w_proj1,
                  w_q2=w_q2, w_k2=w_k2, w_v2=w_v2, w_proj2=w_proj2,
                  b_v1=b_v1, b_v2=b_v2, gamma=gamma, beta=beta)
    f32 = np.float32
    rl = np.asarray(rgb_low, dtype=f32)
    rh = np.asarray(rgb_high, dtype=f32)
    dl = np.asarray(dsm_low, dtype=f32)
    dh = np.asarray(dsm_high, dtype=f32)
    B = rl.shape[0]
    assert B == NCORES, f"expected batch {NCORES}, got {B}"

    wt, wp, bp, bh = _pack_host(inputs)
    nc = _get_nc()

    in_maps = []
    for i in range(NCORES):
        in_maps.append({
            "xq1": np.ascontiguousarray(rl[i].reshape(C, N)),
            "xkv1": np.ascontiguousarray(dh[i].reshape(C, N)),
            "xq2": np.ascontiguousarray(rh[i].reshape(C, N)),
            "xkv2": np.ascontiguousarray(dl[i].reshape(C, N)),
            "wt": wt, "wp": wp, "bp": bp, "bh": bh,
        })

    res = run_bass_kernel_spmd(nc, in_maps, core_ids=list(range(NCORES)),
                               trace=_trace)
    out_low = np.stack([res.results[i]["out1"].reshape(C, 64, 64)
                        for i in range(NCORES)])
    out_high = np.stack([res.results[i]["out2"].reshape(C, 64, 64)
                         for i in range(NCORES)])
    if _trace:
        _CACHE["last_results"] = res
    return (out_low, out_high, np.asarray(dsm_low), np.asarray(dsm_high))
